# revision 14
# baseline (speedup 1.0000x reference)
"""AttentionRNNSLU Trainium2 kernel: bidirectional LSTM encoder + attention
LSTM decoder with argmax feedback, data-parallel over 8 NeuronCores."""
import numpy as np
import concourse.bass as bass
import concourse.bacc as bacc
import concourse.mybir as mybir
from concourse.bass_utils import run_bass_kernel_spmd
from concourse.tile import TileContext

dt = mybir.dt
AF = mybir.ActivationFunctionType
ALU = mybir.AluOpType

B, T, E = 256, 128, 768
HE, H = 256, 512
SLOT, INTENT = 120, 64
EMB = 40
V = 121
NCORES = 8
BC = B // NCORES  # 32 examples per core
NEG = -1e12
WIN = 1055   # alpha window tensor width (33*31 + 32)
QWIN = 284   # q window width (36*7 + 32)

# matmul dtype: float32r = 1 cyc/row (N>=256) reduced-precision fp32;
# float32 = full precision at 4 cyc/row
DT_MM = dt.float32r
F32 = dt.float32

_CACHE = {}


def _winap(t_ap, free_dims, nparts, rowsz):
    """Custom strided AP on an SBUF/PSUM tile: free_dims = [(step, count), ...]."""
    return bass.AP(t_ap.tensor, t_ap.offset,
                   [[rowsz, nparts]] + [[s, c] for s, c in free_dims])


def _build(nc):
    din = nc.dram_tensor

    def ext_in(name, shape, dtype=DT_MM):
        return din(name, list(shape), dtype, kind="ExternalInput").ap()

    # ---- inputs (per-core) ----
    xT = ext_in("xT", [BC, 6, 128, T])            # seq_reps[b].T chunked
    wihT_f = ext_in("wihT_f", [6, 128, 4 * HE])
    wihT_b = ext_in("wihT_b", [6, 128, 4 * HE])
    bias_f = ext_in("bias_f", [1, 4 * HE])
    bias_b = ext_in("bias_b", [1, 4 * HE])
    whhT_f = ext_in("whhT_f", [2, 128, 4 * HE])
    whhT_b = ext_in("whhT_b", [2, 128, 4 * HE])
    # wdx chunks 0-3: Wd_hh.T, 4-7: W2.T, 8: attn_W (4x[128,512] packed)
    wdx = ext_in("wdx", [9, 128, 4 * H])
    tpb = ext_in("tpb", [V + 1, 4 * H])           # [emb@W1.T ; bias] rows
    w3T = ext_in("w3T", [4, 128, 4 * H])
    heads = ext_in("heads", [8, 128, 256])
    headsb = ext_in("headsb", [1, 256])
    identD = ext_in("identD", [128, 128])
    ones1 = ext_in("ones1", [1, 128])
    bos_ohT = ext_in("bos_ohT", [V + 1, BC])
    cnt_m = ext_in("cnt_m", [128, WIN])           # windowed count-onehot.T
    amask = ext_in("amask", [BC, 512], F32)
    iota_f = ext_in("iota_f", [BC, V], F32)

    out_slot = din("out_slot", [BC, T, SLOT], F32, kind="ExternalOutput").ap()
    out_intent = din("out_intent", [BC, INTENT], F32, kind="ExternalOutput").ap()

    # ---- internal DRAM staging ----
    xproj_f = din("xproj_f", [BC, T, 4 * HE], DT_MM).ap()
    xproj_b = din("xproj_b", [BC, T, 4 * HE], DT_MM).ap()
    a3 = din("a3", [T, BC, 4 * H], DT_MM).ap()

    with TileContext(nc) as tc:
        with (
            tc.tile_pool(name="const", bufs=1) as cp,
            tc.tile_pool(name="state", bufs=1) as stp,
        ):
            ident = cp.tile([128, 128], DT_MM)
            nc.sync.dma_start(ident[:], identD[:])
            onesr = cp.tile([1, 128], DT_MM)
            nc.sync.dma_start(onesr[:], ones1[:])
            msk = cp.tile([BC, 512], F32)
            nc.sync.dma_start(msk[:], amask[:])
            iota = cp.tile([BC, V], F32)
            nc.sync.dma_start(iota[:], iota_f[:])
            hds = cp.tile([128, 8, 256], DT_MM)
            nc.sync.dma_start(hds[:], heads[:].rearrange("c p n -> p c n"))
            hdsb = cp.tile([1, 256], DT_MM)
            nc.sync.dma_start(hdsb[:], headsb[:])
            tpbt = cp.tile([V + 1, 4 * H], DT_MM)
            nc.sync.dma_start(tpbt[:], tpb[:])

            # persistent decode state
            hT_d = stp.tile([128, 4, BC], DT_MM)
            ctxT = stp.tile([128, 4, BC], DT_MM)
            ohT = stp.tile([V + 1, BC], DT_MM)
            c_d = stp.tile([BC, H], F32)
            qT_m = stp.tile([128, 4, QWIN], DT_MM)
            alphaT_m = stp.tile([128, WIN], DT_MM)
            nc.gpsimd.memset(qT_m[:].bitcast(dt.uint32), 0)
            nc.gpsimd.memset(hT_d[:].bitcast(dt.uint32), 0)
            nc.gpsimd.memset(alphaT_m[:].bitcast(dt.uint32), 0)
            nc.gpsimd.memset(c_d[:], 0.0)
            nc.sync.dma_start(ohT[:], bos_ohT[:])

            # ================= phase 1: x-projections =================
            with (
                tc.tile_pool(name="p1w", bufs=1) as p1w,
                tc.tile_pool(name="p1x", bufs=3) as p1x,
                tc.tile_pool(name="p1ps", bufs=3, space="PSUM") as p1ps,
            ):
                wf = p1w.tile([128, 6, 4 * HE], DT_MM)
                wb = p1w.tile([128, 6, 4 * HE], DT_MM)
                bf = p1w.tile([1, 4 * HE], DT_MM)
                bb = p1w.tile([1, 4 * HE], DT_MM)
                nc.sync.dma_start(wf[:], wihT_f[:].rearrange("c p n -> p c n"))
                nc.sync.dma_start(wb[:], wihT_b[:].rearrange("c p n -> p c n"))
                nc.sync.dma_start(bf[:], bias_f[:])
                nc.sync.dma_start(bb[:], bias_b[:])
                for b in range(BC):
                    xt = p1x.tile([128, 6, T], DT_MM, tag="xt")
                    nc.sync.dma_start(xt[:], xT[b].rearrange("c p t -> p c t"))
                    for w, bi, xp_d in ((wf, bf, xproj_f), (wb, bb, xproj_b)):
                        pg = p1ps.tile([128, 4 * HE], F32, tag="pg1")
                        for n in range(2):
                            nsl = slice(n * 512, n * 512 + 512)
                            for c in range(6):
                                nc.tensor.matmul(pg[:, nsl], xt[:, c, :], w[:, c, nsl],
                                                 start=(c == 0), stop=False)
                            nc.tensor.matmul(pg[:, nsl], onesr[:], bi[:, nsl],
                                             start=False, stop=True)
                        xps = p1x.tile([128, 4 * HE], DT_MM, tag="xps")
                        nc.vector.tensor_copy(xps[:], pg[:])
                        nc.sync.dma_start(xp_d[b], xps[:])

            # big enc tensors allocated after phase-1 weights are released
            with tc.tile_pool(name="big", bufs=1) as bigp:
                encT = bigp.tile([128, 4, BC, T], DT_MM)    # h-major enc
                enc_sb = bigp.tile([128, BC, H], DT_MM)     # t-major enc

                # ================= phase 2: encoder recurrence =================
                with (
                    tc.tile_pool(name="p2w", bufs=1) as p2w,
                    tc.tile_pool(name="p2s", bufs=1) as p2s,
                    tc.tile_pool(name="p2xp", bufs=3) as p2xp,
                    tc.tile_pool(name="p2t", bufs=2) as p2t,
                    tc.tile_pool(name="p2ps", bufs=2, space="PSUM") as p2ps,
                    tc.tile_pool(name="p2pt", bufs=2, space="PSUM") as p2pt,
                ):
                    whf = p2w.tile([128, 2, 4 * HE], DT_MM)
                    whb = p2w.tile([128, 2, 4 * HE], DT_MM)
                    nc.sync.dma_start(whf[:], whhT_f[:].rearrange("c p n -> p c n"))
                    nc.sync.dma_start(whb[:], whhT_b[:].rearrange("c p n -> p c n"))
                    sts = {}
                    for d in (0, 1):
                        hT_e = p2s.tile([128, 2, BC], DT_MM, tag=f"hTe{d}")
                        c_e = p2s.tile([BC, HE], F32, tag=f"ce{d}")
                        nc.gpsimd.memset(hT_e[:].bitcast(dt.uint32), 0)
                        nc.gpsimd.memset(c_e[:], 0.0)
                        sts[d] = (hT_e, c_e)
                    for s in range(T):
                        for d, wh, xp_d in ((0, whf, xproj_f), (1, whb, xproj_b)):
                            t = s if d == 0 else T - 1 - s
                            hT_e, c_e = sts[d]
                            xp = p2xp.tile([BC, 4 * HE], DT_MM, tag="xp")
                            nc.sync.dma_start(xp[:], xp_d[:, t, :])
                            pg = p2ps.tile([BC, 4 * HE], F32, tag="pg2")
                            for n in range(2):
                                nsl = slice(n * 512, n * 512 + 512)
                                nc.tensor.matmul(pg[:, nsl], ident[:BC, :BC], xp[:, nsl],
                                                 start=True, stop=False)
                                for c in range(2):
                                    nc.tensor.matmul(pg[:, nsl], hT_e[:, c, :],
                                                     wh[:, c, nsl],
                                                     start=False, stop=(c == 1))
                            sg_if = p2t.tile([BC, 2 * HE], F32, tag="sgif")
                            nc.scalar.activation(sg_if[:], pg[:, 0:2 * HE], AF.Sigmoid)
                            tg = p2t.tile([BC, HE], F32, tag="tg")
                            nc.scalar.activation(tg[:], pg[:, 2 * HE:3 * HE], AF.Tanh)
                            so = p2t.tile([BC, HE], F32, tag="so")
                            nc.scalar.activation(so[:], pg[:, 3 * HE:4 * HE], AF.Sigmoid)
                            t1 = p2t.tile([BC, HE], F32, tag="t1")
                            nc.vector.tensor_mul(t1[:], sg_if[:, HE:2 * HE], c_e[:])
                            t2 = p2t.tile([BC, HE], F32, tag="t2")
                            nc.vector.tensor_mul(t2[:], sg_if[:, 0:HE], tg[:])
                            nc.vector.tensor_add(c_e[:], t1[:], t2[:])
                            thc = p2t.tile([BC, HE], F32, tag="tg")
                            nc.scalar.activation(thc[:], c_e[:], AF.Tanh)
                            h_sb = p2t.tile([BC, HE], DT_MM, tag="hsb")
                            nc.vector.tensor_mul(h_sb[:], so[:], thc[:])
                            pt = p2pt.tile([128, 64], DT_MM, tag="pt2")
                            for c in range(2):
                                nc.tensor.transpose(pt[:, c * 32:c * 32 + 32],
                                                    h_sb[:, c * 128:c * 128 + 128],
                                                    ident[:BC, :BC])
                            nc.vector.tensor_copy(
                                hT_e[:], pt[:].rearrange("p (c m) -> p c m", c=2))
                            for c in range(2):
                                nc.vector.tensor_copy(encT[:, 2 * d + c, :, t],
                                                      hT_e[:, c, :])

                # ======== phase 3: build enc_sb (t-major) from encT ========
                with tc.tile_pool(name="p3pt", bufs=6, space="PSUM") as p3pt:
                    for b in range(BC):
                        for c in range(4):
                            pt = p3pt.tile([128, 128], DT_MM, tag="p3")
                            nc.tensor.transpose(pt[:], encT[:, c, b, :], ident[:])
                            nc.vector.tensor_copy(
                                enc_sb[:, b, c * 128:c * 128 + 128], pt[:])

                # ======== phase 4: ctx_init = enc[b, count_b] ========
                with (
                    tc.tile_pool(name="p4", bufs=1) as p4,
                    tc.tile_pool(name="p4ps", bufs=1, space="PSUM") as p4ps,
                    tc.tile_pool(name="p4pt", bufs=1, space="PSUM") as p4pt,
                ):
                    cntm = p4.tile([128, WIN], DT_MM)
                    nc.sync.dma_start(cntm[:], cnt_m[:])
                    pci = p4ps.tile([BC, H], F32)
                    for b in range(BC):
                        nc.tensor.matmul(pci[:, :], cntm[:, 33 * b:33 * b + 32],
                                         enc_sb[:, b, :], start=(b == 0),
                                         stop=(b == BC - 1))
                    ci_sb = p4.tile([BC, H], DT_MM)
                    nc.scalar.activation(ci_sb[:], pci[:], AF.Copy)
                    ptc = p4pt.tile([128, 128], DT_MM)
                    for c in range(4):
                        nc.tensor.transpose(ptc[:, c * 32:c * 32 + 32],
                                            ci_sb[:, c * 128:c * 128 + 128],
                                            ident[:BC, :BC])
                    nc.vector.tensor_copy(ctxT[:], ptc[:].rearrange("p (c m) -> p c m", c=4))

                # ======== phase 5: A3 = enc @ W3.T to DRAM ========
                with (
                    tc.tile_pool(name="p5w", bufs=2) as p5w,
                    tc.tile_pool(name="p5ps", bufs=6, space="PSUM") as p5ps,
                ):
                    for nh in range(2):
                        w3 = p5w.tile([128, 4, 2 * H], DT_MM, tag="w3")
                        nc.sync.dma_start(
                            w3[:], w3T[:, :, nh * 1024:nh * 1024 + 1024]
                            .rearrange("c p n -> p c n"))
                        for b in range(BC):
                            for n in range(2):
                                off = nh * 1024 + n * 512
                                pa = p5ps.tile([128, 512], F32, tag="pa")
                                for c in range(4):
                                    nc.tensor.matmul(
                                        pa[:], encT[:, c, b, :],
                                        w3[:, c, n * 512:n * 512 + 512],
                                        start=(c == 0), stop=(c == 3))
                                pas = p5w.tile([128, 512], DT_MM, tag="pas")
                                nc.vector.tensor_copy(pas[:], pa[:])
                                nc.sync.dma_start(a3[:, b, off:off + 512], pas[:])

                # ================= phase 6: decode loop =================
                with (
                    tc.tile_pool(name="wdxp", bufs=3) as wdxp,
                    tc.tile_pool(name="a3p", bufs=2) as a3p,
                    tc.tile_pool(name="dtp", bufs=1) as dtp,
                    tc.tile_pool(name="dsm", bufs=1) as dsm,
                    tc.tile_pool(name="dps", bufs=2, space="PSUM") as dps,
                    tc.tile_pool(name="dpg", bufs=1, space="PSUM") as dpg,
                    tc.tile_pool(name="dpt", bufs=2, space="PSUM") as dpt,
                ):
                    for t in range(T):
                        # ---- gates ----
                        pg = dpg.tile([BC, 4 * H], F32, tag="pg")
                        for half in range(2):
                            a3t = a3p.tile([BC, 2 * H], DT_MM, tag="a3t")
                            nc.sync.dma_start(
                                a3t[:], a3[t, :, half * 1024:half * 1024 + 1024])
                            for n in range(2):
                                o = half * 1024 + n * 512
                                nc.tensor.matmul(pg[:, o:o + 512], ident[:BC, :BC],
                                                 a3t[:, n * 512:n * 512 + 512],
                                                 start=True, stop=False)
                        for c in range(8):
                            lhs = hT_d[:, c, :] if c < 4 else ctxT[:, c - 4, :]
                            for half in range(2):
                                wt = wdxp.tile([128, 2 * H], DT_MM, tag="wdx")
                                nc.sync.dma_start(
                                    wt[:], wdx[c, :, half * 1024:half * 1024 + 1024])
                                for n in range(2):
                                    o = half * 1024 + n * 512
                                    nc.tensor.matmul(pg[:, o:o + 512], lhs,
                                                     wt[:, n * 512:n * 512 + 512],
                                                     start=False, stop=False)
                        for n in range(4):
                            nc.tensor.matmul(pg[:, n * 512:n * 512 + 512], ohT[:],
                                             tpbt[:, n * 512:n * 512 + 512],
                                             start=False, stop=True)
                        # ---- pointwise ----
                        sg_if = dtp.tile([BC, 2 * H], F32, tag="sgif")
                        nc.scalar.activation(sg_if[:], pg[:, 0:2 * H], AF.Sigmoid)
                        tg = dtp.tile([BC, H], F32, tag="tg")
                        nc.scalar.activation(tg[:], pg[:, 2 * H:3 * H], AF.Tanh)
                        so = dtp.tile([BC, H], F32, tag="so")
                        nc.scalar.activation(so[:], pg[:, 3 * H:4 * H], AF.Sigmoid)
                        t1 = dtp.tile([BC, H], F32, tag="t1")
                        nc.vector.tensor_mul(t1[:], sg_if[:, H:2 * H], c_d[:])
                        t2 = dtp.tile([BC, H], F32, tag="t2")
                        nc.vector.tensor_mul(t2[:], sg_if[:, 0:H], tg[:])
                        nc.vector.tensor_add(c_d[:], t1[:], t2[:])
                        thc = dtp.tile([BC, H], F32, tag="tg")
                        nc.scalar.activation(thc[:], c_d[:], AF.Tanh)
                        h_sb = dtp.tile([BC, H], DT_MM, tag="hsb")
                        nc.vector.tensor_mul(h_sb[:], so[:], thc[:])
                        pth = dpt.tile([128, 128], DT_MM, tag="pth")
                        for c in range(4):
                            nc.tensor.transpose(pth[:, c * 32:c * 32 + 32],
                                                h_sb[:, c * 128:c * 128 + 128],
                                                ident[:BC, :BC])
                        nc.vector.tensor_copy(
                            hT_d[:], pth[:].rearrange("p (c m) -> p c m", c=4))

                        # ---- attention ----
                        pq = dps.tile([BC, 512], F32, tag="pqs")
                        for half in range(2):
                            wq = wdxp.tile([128, 2 * H], DT_MM, tag="wdx")
                            nc.sync.dma_start(
                                wq[:], wdx[8, :, half * 1024:half * 1024 + 1024])
                            for c2 in range(2):
                                c = half * 2 + c2
                                nc.tensor.matmul(pq[:], hT_d[:, c, :],
                                                 wq[:, c2 * 512:c2 * 512 + 512],
                                                 start=(c == 0), stop=(c == 3))
                        q_sb = dtp.tile([BC, H], DT_MM, tag="qsb")
                        nc.scalar.activation(q_sb[:], pq[:], AF.Copy)
                        ptq = dpt.tile([128, 128], DT_MM, tag="pth")
                        for c in range(4):
                            nc.tensor.transpose(ptq[:, c * 32:c * 32 + 32],
                                                q_sb[:, c * 128:c * 128 + 128],
                                                ident[:BC, :BC])
                        ptqv = ptq[:].rearrange("p (c m) -> p c m", c=4)
                        for g in range(8):
                            nc.vector.tensor_copy(qT_m[:, :, 40 * g:40 * g + 4],
                                                  ptqv[:, :, 4 * g:4 * g + 4])
                        pe = dps.tile([BC, 512], F32, tag="pqs")
                        for g in range(8):
                            for c in range(4):
                                nc.tensor.matmul(pe[:], qT_m[:, c, 36 * g:36 * g + 32],
                                                 encT[:, c, 4 * g:4 * g + 4, :],
                                                 start=(g == 0 and c == 0),
                                                 stop=(g == 7 and c == 3))
                        me = dtp.tile([BC, 512], F32, tag="sgif")
                        nc.vector.tensor_add(me[:], pe[:], msk[:])
                        mxa = dsm.tile([BC, 1], F32, tag="mxa")
                        nc.vector.tensor_reduce(out=mxa[:], in_=me[:],
                                                axis=mybir.AxisListType.X, op=ALU.max)
                        ngmx = dsm.tile([BC, 1], F32, tag="ngmx")
                        nc.vector.tensor_scalar_mul(ngmx[:], mxa[:], -1.0)
                        alpha = dtp.tile([BC, 512], DT_MM, tag="tg")
                        asum = dsm.tile([BC, 1], F32, tag="asum")
                        nc.scalar.activation(alpha[:], me[:], AF.Exp, bias=ngmx[:],
                                             scale=1.0, accum_out=asum[:])
                        rsum = dsm.tile([BC, 1], F32, tag="rsum")
                        nc.vector.reciprocal(rsum[:], asum[:])
                        pta = dpt.tile([128, 128], DT_MM, tag="pth")
                        for c in range(4):
                            nc.tensor.transpose(pta[:, c * 32:c * 32 + 32],
                                                alpha[:, c * 128:c * 128 + 128],
                                                ident[:BC, :BC])
                        # alphaT_m[p, 136g+34j] = pta[p, 33j+4g]  (b = 4g+j)
                        nc.vector.tensor_copy(
                            _winap(alphaT_m[:], [(136, 8), (34, 4)], 128, WIN),
                            _winap(pta[:], [(4, 8), (33, 4)], 128, 128))
                        pc = dps.tile([BC, 512], F32, tag="pqs")
                        for b in range(BC):
                            nc.tensor.matmul(pc[:], alphaT_m[:, 33 * b:33 * b + 32],
                                             enc_sb[:, b, :], start=(b == 0),
                                             stop=(b == BC - 1))
                        ctx_sb = dtp.tile([BC, H], DT_MM, tag="so")
                        nc.scalar.activation(ctx_sb[:], pc[:], AF.Copy, scale=rsum[:])
                        ptc = dpt.tile([128, 128], DT_MM, tag="pth")
                        for c in range(4):
                            nc.tensor.transpose(ptc[:, c * 32:c * 32 + 32],
                                                ctx_sb[:, c * 128:c * 128 + 128],
                                                ident[:BC, :BC])
                        # ---- scores (slot head, ctxT = ctx_{t-1}) ----
                        ps = dps.tile([BC, 512], F32, tag="pqs")
                        for c in range(4):
                            nc.tensor.matmul(ps[:, 0:256], hT_d[:, c, :], hds[:, c, :],
                                             start=(c == 0), stop=False)
                        for c in range(4):
                            nc.tensor.matmul(ps[:, 0:256], ctxT[:, c, :],
                                             hds[:, 4 + c, :], start=False, stop=False)
                        nc.tensor.matmul(ps[:, 0:256], onesr[:, :BC], hdsb[:],
                                         start=False, stop=True)
                        scr = dsm.tile([BC, 184], F32, tag="scr")
                        nc.scalar.activation(scr[:], ps[:, 0:184], AF.Copy)
                        # argmax + one-hot (feeds next step's gates)
                        mx8 = dsm.tile([BC, 8], F32, tag="mx8")
                        nc.vector.max(mx8[:], scr[:, 0:SLOT])
                        idx8 = dsm.tile([BC, 8], dt.uint16, tag="idx8")
                        nc.vector.max_index(idx8[:], mx8[:], scr[:, 0:SLOT])
                        idxf = dsm.tile([BC, 1], F32, tag="idxf")
                        nc.vector.tensor_copy(idxf[:], idx8[:, 0:1])
                        oh = dsm.tile([BC, V], DT_MM, tag="oh")
                        nc.vector.tensor_scalar(out=oh[:], in0=iota[:], scalar1=idxf[:],
                                                scalar2=None, op0=ALU.is_equal)
                        poh = dpt.tile([128, 128], DT_MM, tag="pth")
                        nc.tensor.transpose(poh[0:V, 0:BC], oh[:], ident[:BC, :BC])
                        nc.vector.tensor_copy(ohT[0:V, :], poh[0:V, 0:BC])
                        # log-softmax -> slot scores out
                        nmx = dsm.tile([BC, 1], F32, tag="nmx")
                        nc.vector.tensor_scalar_mul(nmx[:], mx8[:, 0:1], -1.0)
                        expv = dtp.tile([BC, H], F32, tag="t2")
                        sume = dsm.tile([BC, 1], F32, tag="sume")
                        nc.scalar.activation(expv[:, 0:SLOT], scr[:, 0:SLOT], AF.Exp,
                                             bias=nmx[:], scale=1.0, accum_out=sume[:])
                        lse = dsm.tile([BC, 1], F32, tag="lse")
                        nc.scalar.activation(lse[:], sume[:], AF.Ln)
                        nb = dsm.tile([BC, 1], F32, tag="nb")
                        nc.vector.tensor_sub(nb[:], nmx[:], lse[:])
                        logp = dtp.tile([BC, H], F32, tag="t1")
                        nc.scalar.activation(logp[:, 0:SLOT], scr[:, 0:SLOT],
                                             AF.Identity, bias=nb[:], scale=1.0)
                        nc.sync.dma_start(out_slot[:, t, :], logp[:, 0:SLOT])

                        nc.vector.tensor_copy(
                            ctxT[:], ptc[:].rearrange("p (c m) -> p c m", c=4))

                        if t == 0:
                            pi = dps.tile([BC, 512], F32, tag="pqs")
                            for c in range(4):
                                nc.tensor.matmul(pi[:, 0:256], hT_d[:, c, :],
                                                 hds[:, c, :], start=(c == 0),
                                                 stop=False)
                            for c in range(4):
                                nc.tensor.matmul(pi[:, 0:256], ctxT[:, c, :],
                                                 hds[:, 4 + c, :],
                                                 start=False, stop=False)
                            nc.tensor.matmul(pi[:, 0:256], onesr[:, :BC], hdsb[:],
                                             start=False, stop=True)
                            intn = dsm.tile([BC, INTENT], F32, tag="intn")
                            nc.scalar.activation(intn[:], pi[:, SLOT:SLOT + INTENT],
                                                 AF.Copy)
                            nc.sync.dma_start(out_intent[:], intn[:])
    nc.compile()
    return nc


def _round_f32r(x):
    # placeholder: host-side rounding to f32r-representable values (identity
    # until HW rounding semantics are confirmed)
    return np.ascontiguousarray(x, dtype=np.float32)


def _prep_core(sl, a):
    f32 = np.float32
    x = np.ascontiguousarray(a["seq_reps"][sl]).astype(f32)       # [BC,T,E]
    xT = np.ascontiguousarray(x.transpose(0, 2, 1)).reshape(BC, 6, 128, T)
    Wd_ih, Wd_hh = a["Wd_ih"], a["Wd_hh"]
    W1 = Wd_ih[:, 0:EMB]
    W2 = Wd_ih[:, EMB:EMB + H]
    W3 = Wd_ih[:, EMB + H:]
    wdx = np.zeros((9, 128, 4 * H), f32)
    wdx[0:4] = Wd_hh.T.reshape(4, 128, 4 * H)
    wdx[4:8] = W2.T.reshape(4, 128, 4 * H)
    # chunk 8: attn_W row-chunks packed side by side [128, 4*512]
    aw = a["attn_W"].reshape(4, 128, H)
    wdx[8] = np.concatenate([aw[c] for c in range(4)], axis=1)
    tpbm = np.concatenate([a["emb_table"] @ W1.T,
                           (a["bd_ih"] + a["bd_hh"])[None, :]], 0)
    hd = np.zeros((1025, 256), f32)
    hd[0:1024, 0:SLOT] = a["slot_W"].T
    hd[0:1024, SLOT:SLOT + INTENT] = a["intent_W"].T
    hd[1024, 0:SLOT] = a["slot_b"]
    hd[1024, SLOT:SLOT + INTENT] = a["intent_b"]
    bos_ohT = np.zeros((V + 1, BC), f32)
    bos_ohT[SLOT, :] = 1.0
    bos_ohT[V, :] = 1.0
    cnt = np.clip((a["nwp_index"][sl, :, 0] != 0).sum(1), 0, T - 1)
    cnt_m = np.zeros((128, WIN), f32)
    for b in range(BC):
        cnt_m[int(cnt[b]), 34 * b] = 1.0
    som = a["slot_output_mask"][sl]                               # [BC,T] bool
    amask = np.full((BC, 512), NEG, f32)
    for b in range(BC):
        j = b % 4
        amask[b, j * 128:(j + 1) * 128] = np.where(som[b], NEG, 0.0)
    r = _round_f32r
    return {
        "xT": r(xT),
        "wihT_f": r(a["Wf_ih"].T.reshape(6, 128, 4 * HE)),
        "wihT_b": r(a["Wb_ih"].T.reshape(6, 128, 4 * HE)),
        "bias_f": r((a["bf_ih"] + a["bf_hh"]).reshape(1, 4 * HE)),
        "bias_b": r((a["bb_ih"] + a["bb_hh"]).reshape(1, 4 * HE)),
        "whhT_f": r(a["Wf_hh"].T.reshape(2, 128, 4 * HE)),
        "whhT_b": r(a["Wb_hh"].T.reshape(2, 128, 4 * HE)),
        "wdx": r(wdx), "tpb": r(tpbm),
        "w3T": r(W3.T.reshape(4, 128, 4 * H)),
        "heads": r(hd[0:1024].reshape(8, 128, 256)),
        "headsb": r(hd[1024:1025]),
        "identD": np.eye(128, dtype=f32),
        "ones1": np.ones((1, 128), f32),
        "bos_ohT": r(bos_ohT), "cnt_m": r(cnt_m),
        "amask": amask,
        "iota_f": np.broadcast_to(np.arange(V, dtype=f32), (BC, V)).copy(),
    }


def get_nc():
    if "nc" not in _CACHE:
        _CACHE["nc"] = _build(bacc.Bacc("TRN2", target_bir_lowering=False, debug=False))
    return _CACHE["nc"]


def make_in_maps(**inputs):
    args = {k: np.asarray(v) for k, v in inputs.items()}
    return [_prep_core(slice(c * BC, (c + 1) * BC), args) for c in range(NCORES)]


def _get_exec():
    if "exec" in _CACHE:
        return _CACHE["exec"]
    import jax
    from jax.sharding import Mesh, PartitionSpec, NamedSharding
    from jax.experimental.shard_map import shard_map
    from concourse.bass2jax import (_bass_exec_p, install_neuronx_cc_hook,
                                    partition_id_tensor)
    nc = get_nc()
    install_neuronx_cc_hook()
    pname = nc.partition_id_tensor.name if nc.partition_id_tensor else None
    in_names, out_names, out_avals, zero_shapes = [], [], [], []
    for alloc in nc.m.functions[0].allocations:
        if not isinstance(alloc, mybir.MemoryLocationSet):
            continue
        name = alloc.memorylocations[0].name
        if alloc.kind == "ExternalInput":
            if name != pname:
                in_names.append(name)
        elif alloc.kind == "ExternalOutput":
            out_names.append(name)
            shape = tuple(alloc.tensor_shape)
            npdt = mybir.dt.np(alloc.dtype)
            out_avals.append(jax.core.ShapedArray(shape, npdt))
            zero_shapes.append((shape, npdt))
    n_params, n_outs = len(in_names), len(out_avals)
    all_in = list(in_names) + out_names + ([pname] if pname else [])

    def _body(*args):
        operands = list(args)
        if pname is not None:
            operands.append(partition_id_tensor())
        return tuple(_bass_exec_p.bind(
            *operands, out_avals=tuple(out_avals), in_names=tuple(all_in),
            out_names=tuple(out_names), lowering_input_output_aliases=(),
            sim_require_finite=True, sim_require_nnan=True, nc=nc))

    mesh = Mesh(np.asarray(jax.devices()[:NCORES]), ("core",))
    sharded = jax.jit(
        shard_map(_body, mesh=mesh,
                  in_specs=(PartitionSpec("core"),) * (n_params + n_outs),
                  out_specs=(PartitionSpec("core"),) * n_outs, check_rep=False),
        donate_argnums=tuple(range(n_params, n_params + n_outs)), keep_unused=True)
    sh = NamedSharding(mesh, PartitionSpec("core"))
    _CACHE["exec"] = (sharded, sh, in_names, out_names, zero_shapes)
    return _CACHE["exec"]


def kernel(**inputs):
    import jax
    sharded, sh, in_names, out_names, zero_shapes = _get_exec()
    in_maps = make_in_maps(**inputs)
    dev_in = [jax.device_put(np.concatenate(
        [np.asarray(in_maps[c][nm]) for c in range(NCORES)], 0), sh)
        for nm in in_names]
    zeros = [jax.device_put(np.zeros((NCORES * s0[0], *s0[1:]), d0), sh)
             for s0, d0 in zero_shapes]
    outs = sharded(*dev_in, *zeros)
    jax.block_until_ready(outs)
    res = {nm: np.asarray(outs[i]) for i, nm in enumerate(out_names)}
    slot_scores = res["out_slot"].reshape(NCORES * BC, T, SLOT)
    intent_score = res["out_intent"].reshape(NCORES * BC, INTENT)
    return slot_scores.astype(np.float32), intent_score.astype(np.float32)


# revision 19
# speedup vs baseline: 1.0036x; 1.0036x over previous
"""AttentionRNNSLU Trainium2 kernel: bidirectional LSTM encoder + attention
LSTM decoder with argmax feedback, data-parallel over 8 NeuronCores."""
import numpy as np
import concourse.bass as bass
import concourse.bacc as bacc
import concourse.mybir as mybir
from concourse.bass_utils import run_bass_kernel_spmd
from concourse.tile import TileContext

dt = mybir.dt
AF = mybir.ActivationFunctionType
ALU = mybir.AluOpType

B, T, E = 256, 128, 768
HE, H = 256, 512
SLOT, INTENT = 120, 64
EMB = 40
V = 121
NCORES = 8
BC = B // NCORES  # 32 examples per core
NEG = -1e12
WIN = 1055   # alpha window tensor width (33*31 + 32)
QWIN = 284   # q window width (36*7 + 32)

# matmul dtype: float32r = 1 cyc/row (N>=256) reduced-precision fp32;
# float32 = full precision at 4 cyc/row
DT_MM = dt.float32r
F32 = dt.float32

_CACHE = {}
PHASES = 99  # model-attribution knob: 1=xproj only, 2=+encoder, 3=+enc_sb, 4=+ctx_init, 5=+A3, 99=full


def _winap(t_ap, free_dims, nparts, rowsz):
    """Custom strided AP on an SBUF/PSUM tile: free_dims = [(step, count), ...]."""
    return bass.AP(t_ap.tensor, t_ap.offset,
                   [[rowsz, nparts]] + [[s, c] for s, c in free_dims])


def _build(nc):
    din = nc.dram_tensor

    def ext_in(name, shape, dtype=DT_MM):
        return din(name, list(shape), dtype, kind="ExternalInput").ap()

    # ---- inputs (per-core) ----
    xT = ext_in("xT", [BC, 6, 128, T])            # seq_reps[b].T chunked
    wihT_f = ext_in("wihT_f", [6, 128, 4 * HE])
    wihT_b = ext_in("wihT_b", [6, 128, 4 * HE])
    bias_f = ext_in("bias_f", [1, 4 * HE])
    bias_b = ext_in("bias_b", [1, 4 * HE])
    whhT_f = ext_in("whhT_f", [2, 128, 4 * HE])
    whhT_b = ext_in("whhT_b", [2, 128, 4 * HE])
    # wdx chunks 0-3: Wd_hh.T, 4-7: W2.T, 8: attn_W (4x[128,512] packed)
    wdx = ext_in("wdx", [9, 128, 4 * H])
    tpb = ext_in("tpb", [V + 1, 4 * H])           # [emb@W1.T ; bias] rows
    w3T = ext_in("w3T", [4, 128, 4 * H])
    heads = ext_in("heads", [8, 128, 256])
    headsb = ext_in("headsb", [1, 256])
    identD = ext_in("identD", [128, 128])
    ones1 = ext_in("ones1", [1, 128])
    bos_ohT = ext_in("bos_ohT", [V + 1, BC])
    cnt_m = ext_in("cnt_m", [128, WIN])           # windowed count-onehot.T
    amask = ext_in("amask", [BC, 512], F32)
    iota_f = ext_in("iota_f", [BC, V], F32)

    out_slot = din("out_slot", [BC, T, SLOT], F32, kind="ExternalOutput").ap()
    out_intent = din("out_intent", [BC, INTENT], F32, kind="ExternalOutput").ap()

    # ---- internal DRAM staging ----
    xproj_f = din("xproj_f", [BC, T, 4 * HE], DT_MM).ap()
    xproj_b = din("xproj_b", [BC, T, 4 * HE], DT_MM).ap()
    a3 = din("a3", [T, BC, 4 * H], DT_MM).ap()

    with TileContext(nc) as tc:
        with (
            tc.tile_pool(name="const", bufs=1) as cp,
            tc.tile_pool(name="state", bufs=1) as stp,
        ):
            ident = cp.tile([128, 128], DT_MM)
            nc.sync.dma_start(ident[:], identD[:])
            onesr = cp.tile([1, 128], DT_MM)
            nc.sync.dma_start(onesr[:], ones1[:])
            msk = cp.tile([BC, 512], F32)
            nc.sync.dma_start(msk[:], amask[:])
            iota = cp.tile([BC, V], F32)
            nc.sync.dma_start(iota[:], iota_f[:])
            hds = cp.tile([128, 8, 256], DT_MM)
            nc.sync.dma_start(hds[:], heads[:].rearrange("c p n -> p c n"))
            hdsb = cp.tile([1, 256], DT_MM)
            nc.sync.dma_start(hdsb[:], headsb[:])
            tpbt = cp.tile([V + 1, 4 * H], DT_MM)
            nc.sync.dma_start(tpbt[:], tpb[:])

            # persistent decode state
            hT_d = stp.tile([128, 4, BC], DT_MM)
            ctxT = stp.tile([128, 4, BC], DT_MM)
            ohT = stp.tile([V + 1, BC], DT_MM)
            c_d = stp.tile([BC, H], F32)
            qT_m = stp.tile([128, 4, QWIN], DT_MM)
            alphaT_m = stp.tile([128, WIN], DT_MM)
            nc.gpsimd.memset(qT_m[:].bitcast(dt.uint32), 0)
            nc.gpsimd.memset(hT_d[:].bitcast(dt.uint32), 0)
            nc.gpsimd.memset(alphaT_m[:].bitcast(dt.uint32), 0)
            nc.gpsimd.memset(c_d[:], 0.0)
            nc.sync.dma_start(ohT[:], bos_ohT[:])

            # ================= phase 1: x-projections =================
            with (
                tc.tile_pool(name="p1w", bufs=1) as p1w,
                tc.tile_pool(name="p1x", bufs=3) as p1x,
                tc.tile_pool(name="p1ps", bufs=3, space="PSUM") as p1ps,
            ):
                wf = p1w.tile([128, 6, 4 * HE], DT_MM)
                wb = p1w.tile([128, 6, 4 * HE], DT_MM)
                bf = p1w.tile([1, 4 * HE], DT_MM)
                bb = p1w.tile([1, 4 * HE], DT_MM)
                nc.sync.dma_start(wf[:], wihT_f[:].rearrange("c p n -> p c n"))
                nc.sync.dma_start(wb[:], wihT_b[:].rearrange("c p n -> p c n"))
                nc.sync.dma_start(bf[:], bias_f[:])
                nc.sync.dma_start(bb[:], bias_b[:])
                for b in range(BC):
                    xt = p1x.tile([128, 6, T], DT_MM, tag="xt")
                    nc.sync.dma_start(xt[:], xT[b].rearrange("c p t -> p c t"))
                    for w, bi, xp_d in ((wf, bf, xproj_f), (wb, bb, xproj_b)):
                        pg = p1ps.tile([128, 4 * HE], F32, tag="pg1")
                        for n in range(2):
                            nsl = slice(n * 512, n * 512 + 512)
                            for c in range(6):
                                nc.tensor.matmul(pg[:, nsl], xt[:, c, :], w[:, c, nsl],
                                                 start=(c == 0), stop=False)
                            nc.tensor.matmul(pg[:, nsl], onesr[:], bi[:, nsl],
                                             start=False, stop=True)
                        xps = p1x.tile([128, 4 * HE], DT_MM, tag="xps")
                        nc.vector.tensor_copy(xps[:], pg[:])
                        nc.sync.dma_start(xp_d[b], xps[:])

            # big enc tensors allocated after phase-1 weights are released
            if PHASES < 2:
                nc.compile()
                return nc
            with tc.tile_pool(name="big", bufs=1) as bigp:
                encT = bigp.tile([128, 4, BC, T], DT_MM)    # h-major enc
                enc_sb = bigp.tile([128, BC, H], DT_MM)     # t-major enc

                # ================= phase 2: encoder recurrence =================
                with (
                    tc.tile_pool(name="p2w", bufs=1) as p2w,
                    tc.tile_pool(name="p2s", bufs=1) as p2s,
                    tc.tile_pool(name="p2xp", bufs=3) as p2xp,
                    tc.tile_pool(name="p2t", bufs=2) as p2t,
                    tc.tile_pool(name="p2ps", bufs=2, space="PSUM") as p2ps,
                    tc.tile_pool(name="p2pt", bufs=2, space="PSUM") as p2pt,
                ):
                    whf = p2w.tile([128, 2, 4 * HE], DT_MM)
                    whb = p2w.tile([128, 2, 4 * HE], DT_MM)
                    nc.sync.dma_start(whf[:], whhT_f[:].rearrange("c p n -> p c n"))
                    nc.sync.dma_start(whb[:], whhT_b[:].rearrange("c p n -> p c n"))
                    sts = {}
                    for d in (0, 1):
                        hT_e = p2s.tile([128, 2, BC], DT_MM, tag=f"hTe{d}")
                        c_e = p2s.tile([BC, HE], F32, tag=f"ce{d}")
                        nc.gpsimd.memset(hT_e[:].bitcast(dt.uint32), 0)
                        nc.gpsimd.memset(c_e[:], 0.0)
                        sts[d] = (hT_e, c_e)
                    for s in range(T):
                        for d, wh, xp_d in ((0, whf, xproj_f), (1, whb, xproj_b)):
                            t = s if d == 0 else T - 1 - s
                            hT_e, c_e = sts[d]
                            xp = p2xp.tile([BC, 4 * HE], DT_MM, tag="xp")
                            nc.sync.dma_start(xp[:], xp_d[:, t, :])
                            pg = p2ps.tile([BC, 4 * HE], F32, tag="pg2")
                            for n in range(2):
                                nsl = slice(n * 512, n * 512 + 512)
                                nc.tensor.matmul(pg[:, nsl], ident[:BC, :BC], xp[:, nsl],
                                                 start=True, stop=False)
                                for c in range(2):
                                    nc.tensor.matmul(pg[:, nsl], hT_e[:, c, :],
                                                     wh[:, c, nsl],
                                                     start=False, stop=(c == 1))
                            sg_if = p2t.tile([BC, 2 * HE], F32, tag="sgif")
                            nc.scalar.activation(sg_if[:], pg[:, 0:2 * HE], AF.Sigmoid)
                            tg = p2t.tile([BC, HE], F32, tag="tg")
                            nc.scalar.activation(tg[:], pg[:, 2 * HE:3 * HE], AF.Tanh)
                            so = p2t.tile([BC, HE], F32, tag="so")
                            nc.scalar.activation(so[:], pg[:, 3 * HE:4 * HE], AF.Sigmoid)
                            t1 = p2t.tile([BC, HE], F32, tag="t1")
                            nc.vector.tensor_mul(t1[:], sg_if[:, HE:2 * HE], c_e[:])
                            t2 = p2t.tile([BC, HE], F32, tag="t2")
                            nc.vector.tensor_mul(t2[:], sg_if[:, 0:HE], tg[:])
                            nc.vector.tensor_add(c_e[:], t1[:], t2[:])
                            thc = p2t.tile([BC, HE], F32, tag="tg")
                            nc.scalar.activation(thc[:], c_e[:], AF.Tanh)
                            h_sb = p2t.tile([BC, HE], DT_MM, tag="hsb")
                            nc.vector.tensor_mul(h_sb[:], so[:], thc[:])
                            pt = p2pt.tile([128, 64], DT_MM, tag="pt2")
                            for c in range(2):
                                nc.tensor.transpose(pt[:, c * 32:c * 32 + 32],
                                                    h_sb[:, c * 128:c * 128 + 128],
                                                    ident[:BC, :BC])
                            nc.vector.tensor_copy(
                                hT_e[:], pt[:].rearrange("p (c m) -> p c m", c=2))
                            for c in range(2):
                                nc.vector.tensor_copy(encT[:, 2 * d + c, :, t],
                                                      hT_e[:, c, :])

                # ======== phase 3: build enc_sb (t-major) from encT ========
                with tc.tile_pool(name="p3pt", bufs=6, space="PSUM") as p3pt:
                  if PHASES >= 3:
                    for b in range(BC):
                        for c in range(4):
                            pt = p3pt.tile([128, 128], DT_MM, tag="p3")
                            nc.tensor.transpose(pt[:], encT[:, c, b, :], ident[:])
                            nc.vector.tensor_copy(
                                enc_sb[:, b, c * 128:c * 128 + 128], pt[:])

                if PHASES < 4:
                    nc.compile()
                    return nc
                # ======== phase 4: ctx_init = enc[b, count_b] ========
                with (
                    tc.tile_pool(name="p4", bufs=1) as p4,
                    tc.tile_pool(name="p4ps", bufs=1, space="PSUM") as p4ps,
                    tc.tile_pool(name="p4pt", bufs=1, space="PSUM") as p4pt,
                ):
                    cntm = p4.tile([128, WIN], DT_MM)
                    nc.sync.dma_start(cntm[:], cnt_m[:])
                    pci = p4ps.tile([BC, H], F32)
                    for b in range(BC):
                        nc.tensor.matmul(pci[:, :], cntm[:, 33 * b:33 * b + 32],
                                         enc_sb[:, b, :], start=(b == 0),
                                         stop=(b == BC - 1))
                    ci_sb = p4.tile([BC, H], DT_MM)
                    nc.scalar.activation(ci_sb[:], pci[:], AF.Copy)
                    ptc = p4pt.tile([128, 128], DT_MM)
                    for c in range(4):
                        nc.tensor.transpose(ptc[:, c * 32:c * 32 + 32],
                                            ci_sb[:, c * 128:c * 128 + 128],
                                            ident[:BC, :BC])
                    nc.vector.tensor_copy(ctxT[:], ptc[:].rearrange("p (c m) -> p c m", c=4))

                if PHASES < 5:
                    nc.compile()
                    return nc
                # ======== phase 5: A3 = enc @ W3.T to DRAM ========
                with (
                    tc.tile_pool(name="p5w", bufs=2) as p5w,
                    tc.tile_pool(name="p5ps", bufs=6, space="PSUM") as p5ps,
                ):
                    for nh in range(2):
                        w3 = p5w.tile([128, 4, 2 * H], DT_MM, tag="w3")
                        nc.sync.dma_start(
                            w3[:], w3T[:, :, nh * 1024:nh * 1024 + 1024]
                            .rearrange("c p n -> p c n"))
                        for b in range(BC):
                            for n in range(2):
                                off = nh * 1024 + n * 512
                                pa = p5ps.tile([128, 512], F32, tag="pa")
                                for c in range(4):
                                    nc.tensor.matmul(
                                        pa[:], encT[:, c, b, :],
                                        w3[:, c, n * 512:n * 512 + 512],
                                        start=(c == 0), stop=(c == 3))
                                pas = p5w.tile([128, 512], DT_MM, tag="pas")
                                nc.vector.tensor_copy(pas[:], pa[:])
                                nc.sync.dma_start(a3[:, b, off:off + 512], pas[:])

                if PHASES < 6:
                    nc.compile()
                    return nc
                # ================= phase 6: decode loop =================
                with (
                    tc.tile_pool(name="wdxp", bufs=3) as wdxp,
                    tc.tile_pool(name="a3p", bufs=2) as a3p,
                    tc.tile_pool(name="dtp", bufs=1) as dtp,
                    tc.tile_pool(name="dsm", bufs=1) as dsm,
                    tc.tile_pool(name="dps", bufs=2, space="PSUM") as dps,
                    tc.tile_pool(name="dpg", bufs=1, space="PSUM") as dpg,
                    tc.tile_pool(name="dpt", bufs=2, space="PSUM") as dpt,
                ):
                    for t in range(T):
                        # ---- gates ----
                        pg = dpg.tile([BC, 4 * H], F32, tag="pg")
                        for half in range(2):
                            a3t = a3p.tile([BC, 2 * H], DT_MM, tag="a3t")
                            nc.sync.dma_start(
                                a3t[:], a3[t, :, half * 1024:half * 1024 + 1024])
                            for n in range(2):
                                o = half * 1024 + n * 512
                                nc.tensor.matmul(pg[:, o:o + 512], ident[:BC, :BC],
                                                 a3t[:, n * 512:n * 512 + 512],
                                                 start=True, stop=False)
                        for c in range(8):
                            lhs = hT_d[:, c, :] if c < 4 else ctxT[:, c - 4, :]
                            for half in range(2):
                                wt = wdxp.tile([128, 2 * H], DT_MM, tag="wdx")
                                nc.sync.dma_start(
                                    wt[:], wdx[c, :, half * 1024:half * 1024 + 1024])
                                for n in range(2):
                                    o = half * 1024 + n * 512
                                    nc.tensor.matmul(pg[:, o:o + 512], lhs,
                                                     wt[:, n * 512:n * 512 + 512],
                                                     start=False, stop=False)
                        for n in range(4):
                            nc.tensor.matmul(pg[:, n * 512:n * 512 + 512], ohT[:],
                                             tpbt[:, n * 512:n * 512 + 512],
                                             start=False, stop=True)
                        # ---- pointwise ----
                        sg_if = dtp.tile([BC, 2 * H], F32, tag="sgif")
                        nc.scalar.activation(sg_if[:], pg[:, 0:2 * H], AF.Sigmoid)
                        tg = dtp.tile([BC, H], F32, tag="tg")
                        nc.scalar.activation(tg[:], pg[:, 2 * H:3 * H], AF.Tanh)
                        so = dtp.tile([BC, H], F32, tag="so")
                        nc.scalar.activation(so[:], pg[:, 3 * H:4 * H], AF.Sigmoid)
                        t1 = dtp.tile([BC, H], F32, tag="t1")
                        nc.vector.tensor_mul(t1[:], sg_if[:, H:2 * H], c_d[:])
                        t2 = dtp.tile([BC, H], F32, tag="t2")
                        nc.vector.tensor_mul(t2[:], sg_if[:, 0:H], tg[:])
                        nc.vector.tensor_add(c_d[:], t1[:], t2[:])
                        thc = dtp.tile([BC, H], F32, tag="tg")
                        nc.scalar.activation(thc[:], c_d[:], AF.Tanh)
                        h_sb = dtp.tile([BC, H], DT_MM, tag="hsb")
                        nc.vector.tensor_mul(h_sb[:], so[:], thc[:])
                        pth = dpt.tile([128, 128], DT_MM, tag="pth")
                        for c in range(4):
                            nc.tensor.transpose(pth[:, c * 32:c * 32 + 32],
                                                h_sb[:, c * 128:c * 128 + 128],
                                                ident[:BC, :BC])
                        nc.vector.tensor_copy(
                            hT_d[:], pth[:].rearrange("p (c m) -> p c m", c=4))

                        # ---- attention ----
                        pq = dps.tile([BC, 512], F32, tag="pqs")
                        for half in range(2):
                            wq = wdxp.tile([128, 2 * H], DT_MM, tag="wdx")
                            nc.sync.dma_start(
                                wq[:], wdx[8, :, half * 1024:half * 1024 + 1024])
                            for c2 in range(2):
                                c = half * 2 + c2
                                nc.tensor.matmul(pq[:], hT_d[:, c, :],
                                                 wq[:, c2 * 512:c2 * 512 + 512],
                                                 start=(c == 0), stop=(c == 3))
                        q_sb = dtp.tile([BC, H], DT_MM, tag="qsb")
                        nc.scalar.activation(q_sb[:], pq[:], AF.Copy)
                        ptq = dpt.tile([128, 128], DT_MM, tag="pth")
                        for c in range(4):
                            nc.tensor.transpose(ptq[:, c * 32:c * 32 + 32],
                                                q_sb[:, c * 128:c * 128 + 128],
                                                ident[:BC, :BC])
                        ptqv = ptq[:].rearrange("p (c m) -> p c m", c=4)
                        for g in range(8):
                            nc.vector.tensor_copy(qT_m[:, :, 40 * g:40 * g + 4],
                                                  ptqv[:, :, 4 * g:4 * g + 4])
                        pe = dps.tile([BC, 512], F32, tag="pqs")
                        for g in range(8):
                            for c in range(4):
                                nc.tensor.matmul(pe[:], qT_m[:, c, 36 * g:36 * g + 32],
                                                 encT[:, c, 4 * g:4 * g + 4, :],
                                                 start=(g == 0 and c == 0),
                                                 stop=(g == 7 and c == 3))
                        me = dtp.tile([BC, 512], F32, tag="sgif")
                        nc.vector.tensor_add(me[:], pe[:], msk[:])
                        mxa = dsm.tile([BC, 1], F32, tag="mxa")
                        nc.vector.tensor_reduce(out=mxa[:], in_=me[:],
                                                axis=mybir.AxisListType.X, op=ALU.max)
                        ngmx = dsm.tile([BC, 1], F32, tag="ngmx")
                        nc.vector.tensor_scalar_mul(ngmx[:], mxa[:], -1.0)
                        alpha = dtp.tile([BC, 512], DT_MM, tag="tg")
                        asum = dsm.tile([BC, 1], F32, tag="asum")
                        nc.scalar.activation(alpha[:], me[:], AF.Exp, bias=ngmx[:],
                                             scale=1.0, accum_out=asum[:])
                        rsum = dsm.tile([BC, 1], F32, tag="rsum")
                        nc.vector.reciprocal(rsum[:], asum[:])
                        pta = dpt.tile([128, 128], DT_MM, tag="pth")
                        for c in range(4):
                            nc.tensor.transpose(pta[:, c * 32:c * 32 + 32],
                                                alpha[:, c * 128:c * 128 + 128],
                                                ident[:BC, :BC])
                        # alphaT_m[p, 136g+34j] = pta[p, 33j+4g]  (b = 4g+j)
                        nc.vector.tensor_copy(
                            _winap(alphaT_m[:], [(136, 8), (34, 4)], 128, WIN),
                            _winap(pta[:], [(4, 8), (33, 4)], 128, 128))
                        pc = dps.tile([BC, 512], F32, tag="pqs")
                        for b in range(BC):
                            nc.tensor.matmul(pc[:], alphaT_m[:, 33 * b:33 * b + 32],
                                             enc_sb[:, b, :], start=(b == 0),
                                             stop=(b == BC - 1))
                        ctx_sb = dtp.tile([BC, H], DT_MM, tag="so")
                        nc.scalar.activation(ctx_sb[:], pc[:], AF.Copy, scale=rsum[:])
                        ptc = dpt.tile([128, 128], DT_MM, tag="pth")
                        for c in range(4):
                            nc.tensor.transpose(ptc[:, c * 32:c * 32 + 32],
                                                ctx_sb[:, c * 128:c * 128 + 128],
                                                ident[:BC, :BC])
                        # ---- scores (slot head, ctxT = ctx_{t-1}) ----
                        ps = dps.tile([BC, 512], F32, tag="pqs")
                        for c in range(4):
                            nc.tensor.matmul(ps[:, 0:256], hT_d[:, c, :], hds[:, c, :],
                                             start=(c == 0), stop=False)
                        for c in range(4):
                            nc.tensor.matmul(ps[:, 0:256], ctxT[:, c, :],
                                             hds[:, 4 + c, :], start=False, stop=False)
                        nc.tensor.matmul(ps[:, 0:256], onesr[:, :BC], hdsb[:],
                                         start=False, stop=True)
                        scr = dsm.tile([BC, 184], F32, tag="scr")
                        nc.scalar.activation(scr[:], ps[:, 0:184], AF.Copy)
                        # argmax + one-hot (feeds next step's gates)
                        mx8 = dsm.tile([BC, 8], F32, tag="mx8")
                        nc.vector.max(mx8[:], scr[:, 0:SLOT])
                        idx8 = dsm.tile([BC, 8], dt.uint16, tag="idx8")
                        nc.vector.max_index(idx8[:], mx8[:], scr[:, 0:SLOT])
                        idxf = dsm.tile([BC, 1], F32, tag="idxf")
                        nc.vector.tensor_copy(idxf[:], idx8[:, 0:1])
                        oh = dsm.tile([BC, V], DT_MM, tag="oh")
                        nc.vector.tensor_scalar(out=oh[:], in0=iota[:], scalar1=idxf[:],
                                                scalar2=None, op0=ALU.is_equal)
                        poh = dpt.tile([128, 128], DT_MM, tag="pth")
                        nc.tensor.transpose(poh[0:V, 0:BC], oh[:], ident[:BC, :BC])
                        nc.vector.tensor_copy(ohT[0:V, :], poh[0:V, 0:BC])
                        # log-softmax -> slot scores out
                        nmx = dsm.tile([BC, 1], F32, tag="nmx")
                        nc.vector.tensor_scalar_mul(nmx[:], mx8[:, 0:1], -1.0)
                        expv = dtp.tile([BC, H], F32, tag="t2")
                        sume = dsm.tile([BC, 1], F32, tag="sume")
                        nc.scalar.activation(expv[:, 0:SLOT], scr[:, 0:SLOT], AF.Exp,
                                             bias=nmx[:], scale=1.0, accum_out=sume[:])
                        lse = dsm.tile([BC, 1], F32, tag="lse")
                        nc.scalar.activation(lse[:], sume[:], AF.Ln)
                        nb = dsm.tile([BC, 1], F32, tag="nb")
                        nc.vector.tensor_sub(nb[:], nmx[:], lse[:])
                        logp = dtp.tile([BC, H], F32, tag="t1")
                        nc.scalar.activation(logp[:, 0:SLOT], scr[:, 0:SLOT],
                                             AF.Identity, bias=nb[:], scale=1.0)
                        nc.sync.dma_start(out_slot[:, t, :], logp[:, 0:SLOT])

                        nc.vector.tensor_copy(
                            ctxT[:], ptc[:].rearrange("p (c m) -> p c m", c=4))

                        if t == 0:
                            pi = dps.tile([BC, 512], F32, tag="pqs")
                            for c in range(4):
                                nc.tensor.matmul(pi[:, 0:256], hT_d[:, c, :],
                                                 hds[:, c, :], start=(c == 0),
                                                 stop=False)
                            for c in range(4):
                                nc.tensor.matmul(pi[:, 0:256], ctxT[:, c, :],
                                                 hds[:, 4 + c, :],
                                                 start=False, stop=False)
                            nc.tensor.matmul(pi[:, 0:256], onesr[:, :BC], hdsb[:],
                                             start=False, stop=True)
                            intn = dsm.tile([BC, INTENT], F32, tag="intn")
                            nc.scalar.activation(intn[:], pi[:, SLOT:SLOT + INTENT],
                                                 AF.Copy)
                            nc.sync.dma_start(out_intent[:], intn[:])
    nc.compile()
    return nc


def _round_f32r(x):
    # placeholder: host-side rounding to f32r-representable values (identity
    # until HW rounding semantics are confirmed)
    return np.ascontiguousarray(x, dtype=np.float32)


def _prep_core(sl, a):
    f32 = np.float32
    x = np.ascontiguousarray(a["seq_reps"][sl]).astype(f32)       # [BC,T,E]
    xT = np.ascontiguousarray(x.transpose(0, 2, 1)).reshape(BC, 6, 128, T)
    Wd_ih, Wd_hh = a["Wd_ih"], a["Wd_hh"]
    W1 = Wd_ih[:, 0:EMB]
    W2 = Wd_ih[:, EMB:EMB + H]
    W3 = Wd_ih[:, EMB + H:]
    wdx = np.zeros((9, 128, 4 * H), f32)
    wdx[0:4] = Wd_hh.T.reshape(4, 128, 4 * H)
    wdx[4:8] = W2.T.reshape(4, 128, 4 * H)
    # chunk 8: attn_W row-chunks packed side by side [128, 4*512]
    aw = a["attn_W"].reshape(4, 128, H)
    wdx[8] = np.concatenate([aw[c] for c in range(4)], axis=1)
    tpbm = np.concatenate([a["emb_table"] @ W1.T,
                           (a["bd_ih"] + a["bd_hh"])[None, :]], 0)
    hd = np.zeros((1025, 256), f32)
    hd[0:1024, 0:SLOT] = a["slot_W"].T
    hd[0:1024, SLOT:SLOT + INTENT] = a["intent_W"].T
    hd[1024, 0:SLOT] = a["slot_b"]
    hd[1024, SLOT:SLOT + INTENT] = a["intent_b"]
    bos_ohT = np.zeros((V + 1, BC), f32)
    bos_ohT[SLOT, :] = 1.0
    bos_ohT[V, :] = 1.0
    cnt = np.clip((a["nwp_index"][sl, :, 0] != 0).sum(1), 0, T - 1)
    cnt_m = np.zeros((128, WIN), f32)
    for b in range(BC):
        cnt_m[int(cnt[b]), 34 * b] = 1.0
    som = a["slot_output_mask"][sl]                               # [BC,T] bool
    amask = np.full((BC, 512), NEG, f32)
    for b in range(BC):
        j = b % 4
        amask[b, j * 128:(j + 1) * 128] = np.where(som[b], NEG, 0.0)
    r = _round_f32r
    return {
        "xT": r(xT),
        "wihT_f": r(a["Wf_ih"].T.reshape(6, 128, 4 * HE)),
        "wihT_b": r(a["Wb_ih"].T.reshape(6, 128, 4 * HE)),
        "bias_f": r((a["bf_ih"] + a["bf_hh"]).reshape(1, 4 * HE)),
        "bias_b": r((a["bb_ih"] + a["bb_hh"]).reshape(1, 4 * HE)),
        "whhT_f": r(a["Wf_hh"].T.reshape(2, 128, 4 * HE)),
        "whhT_b": r(a["Wb_hh"].T.reshape(2, 128, 4 * HE)),
        "wdx": r(wdx), "tpb": r(tpbm),
        "w3T": r(W3.T.reshape(4, 128, 4 * H)),
        "heads": r(hd[0:1024].reshape(8, 128, 256)),
        "headsb": r(hd[1024:1025]),
        "identD": np.eye(128, dtype=f32),
        "ones1": np.ones((1, 128), f32),
        "bos_ohT": r(bos_ohT), "cnt_m": r(cnt_m),
        "amask": amask,
        "iota_f": np.broadcast_to(np.arange(V, dtype=f32), (BC, V)).copy(),
    }


def get_nc():
    if "nc" not in _CACHE:
        _CACHE["nc"] = _build(bacc.Bacc("TRN2", target_bir_lowering=False, debug=False))
    return _CACHE["nc"]


def make_in_maps(**inputs):
    args = {k: np.asarray(v) for k, v in inputs.items()}
    return [_prep_core(slice(c * BC, (c + 1) * BC), args) for c in range(NCORES)]


def _get_exec():
    if "exec" in _CACHE:
        return _CACHE["exec"]
    import jax
    from jax.sharding import Mesh, PartitionSpec, NamedSharding
    from jax.experimental.shard_map import shard_map
    from concourse.bass2jax import (_bass_exec_p, install_neuronx_cc_hook,
                                    partition_id_tensor)
    nc = get_nc()
    install_neuronx_cc_hook()
    pname = nc.partition_id_tensor.name if nc.partition_id_tensor else None
    in_names, out_names, out_avals, zero_shapes = [], [], [], []
    for alloc in nc.m.functions[0].allocations:
        if not isinstance(alloc, mybir.MemoryLocationSet):
            continue
        name = alloc.memorylocations[0].name
        if alloc.kind == "ExternalInput":
            if name != pname:
                in_names.append(name)
        elif alloc.kind == "ExternalOutput":
            out_names.append(name)
            shape = tuple(alloc.tensor_shape)
            npdt = mybir.dt.np(alloc.dtype)
            out_avals.append(jax.core.ShapedArray(shape, npdt))
            zero_shapes.append((shape, npdt))
    n_params, n_outs = len(in_names), len(out_avals)
    all_in = list(in_names) + out_names + ([pname] if pname else [])

    def _body(*args):
        operands = list(args)
        if pname is not None:
            operands.append(partition_id_tensor())
        return tuple(_bass_exec_p.bind(
            *operands, out_avals=tuple(out_avals), in_names=tuple(all_in),
            out_names=tuple(out_names), lowering_input_output_aliases=(),
            sim_require_finite=True, sim_require_nnan=True, nc=nc))

    mesh = Mesh(np.asarray(jax.devices()[:NCORES]), ("core",))
    sharded = jax.jit(
        shard_map(_body, mesh=mesh,
                  in_specs=(PartitionSpec("core"),) * (n_params + n_outs),
                  out_specs=(PartitionSpec("core"),) * n_outs, check_rep=False),
        donate_argnums=tuple(range(n_params, n_params + n_outs)), keep_unused=True)
    sh = NamedSharding(mesh, PartitionSpec("core"))
    _CACHE["exec"] = (sharded, sh, in_names, out_names, zero_shapes)
    return _CACHE["exec"]


def kernel(**inputs):
    import jax
    sharded, sh, in_names, out_names, zero_shapes = _get_exec()
    in_maps = make_in_maps(**inputs)
    dev_in = [jax.device_put(np.concatenate(
        [np.asarray(in_maps[c][nm]) for c in range(NCORES)], 0), sh)
        for nm in in_names]
    zeros = [jax.device_put(np.zeros((NCORES * s0[0], *s0[1:]), d0), sh)
             for s0, d0 in zero_shapes]
    outs = sharded(*dev_in, *zeros)
    jax.block_until_ready(outs)
    res = {nm: np.asarray(outs[i]) for i, nm in enumerate(out_names)}
    slot_scores = res["out_slot"].reshape(NCORES * BC, T, SLOT)
    intent_score = res["out_intent"].reshape(NCORES * BC, INTENT)
    return slot_scores.astype(np.float32), intent_score.astype(np.float32)


# revision 23
# speedup vs baseline: 1.1910x; 1.1867x over previous
"""AttentionRNNSLU Trainium2 kernel: bidirectional LSTM encoder + attention
LSTM decoder with argmax feedback, data-parallel over 8 NeuronCores."""
import numpy as np
import concourse.bass as bass
import concourse.bacc as bacc
import concourse.mybir as mybir
from concourse.bass_utils import run_bass_kernel_spmd
from concourse.tile import TileContext

dt = mybir.dt
AF = mybir.ActivationFunctionType
ALU = mybir.AluOpType

B, T, E = 256, 128, 768
HE, H = 256, 512
SLOT, INTENT = 120, 64
EMB = 40
V = 121
NCORES = 8
BC = B // NCORES  # 32 examples per core
NEG = -1e12
WIN = 1055   # alpha window tensor width (33*31 + 32)
QWIN = 284   # q window width (36*7 + 32)

# matmul dtype: float32r = 1 cyc/row (N>=256) reduced-precision fp32;
# float32 = full precision at 4 cyc/row
DT_MM = dt.float32r
F32 = dt.float32

_CACHE = {}
PHASES = 99  # model-attribution knob: 1=xproj only, 2=+encoder, 3=+enc_sb, 4=+ctx_init, 5=+A3, 99=full


def _winap(t_ap, free_dims, nparts, rowsz):
    """Custom strided AP on an SBUF/PSUM tile: free_dims = [(step, count), ...]."""
    return bass.AP(t_ap.tensor, t_ap.offset,
                   [[rowsz, nparts]] + [[s, c] for s, c in free_dims])


def _build(nc):
    din = nc.dram_tensor

    def ext_in(name, shape, dtype=DT_MM):
        return din(name, list(shape), dtype, kind="ExternalInput").ap()

    # ---- inputs (per-core) ----
    xT = ext_in("xT", [BC, 6, 128, T])            # seq_reps[b].T chunked
    wihT_f = ext_in("wihT_f", [6, 128, 4 * HE])
    wihT_b = ext_in("wihT_b", [6, 128, 4 * HE])
    bias_f = ext_in("bias_f", [1, 4 * HE])
    bias_b = ext_in("bias_b", [1, 4 * HE])
    whhT_f = ext_in("whhT_f", [2, 128, 4 * HE])
    whhT_b = ext_in("whhT_b", [2, 128, 4 * HE])
    # wdx chunks 0-3: Wd_hh.T, 4-7: W2.T, 8: attn_W (4x[128,512] packed)
    wdx = ext_in("wdx", [9, 128, 4 * H])
    tpb = ext_in("tpb", [V + 1, 4 * H])           # [emb@W1.T ; bias] rows
    w3T = ext_in("w3T", [4, 128, 4 * H])
    heads = ext_in("heads", [8, 128, 256])
    headsb = ext_in("headsb", [1, 256])
    identD = ext_in("identD", [128, 128])
    ones1 = ext_in("ones1", [1, 128])
    bos_ohT = ext_in("bos_ohT", [V + 1, BC])
    cnt_m = ext_in("cnt_m", [128, WIN])           # windowed count-onehot.T
    amask = ext_in("amask", [BC, 512], F32)
    iota_f = ext_in("iota_f", [BC, V], F32)

    out_slot = din("out_slot", [BC, T, SLOT], F32, kind="ExternalOutput").ap()
    out_intent = din("out_intent", [BC, INTENT], F32, kind="ExternalOutput").ap()

    # ---- internal DRAM staging ----
    xproj_f = din("xproj_f", [BC, T, 4 * HE], DT_MM).ap()
    xproj_b = din("xproj_b", [BC, T, 4 * HE], DT_MM).ap()
    a3 = din("a3", [T, BC, 4 * H], DT_MM).ap()

    with TileContext(nc) as tc:
        with (
            tc.tile_pool(name="const", bufs=1) as cp,
            tc.tile_pool(name="state", bufs=1) as stp,
        ):
            ident = cp.tile([128, 128], DT_MM)
            nc.sync.dma_start(ident[:], identD[:])
            onesr = cp.tile([1, 128], DT_MM)
            nc.sync.dma_start(onesr[:], ones1[:])
            msk = cp.tile([BC, 512], F32)
            nc.sync.dma_start(msk[:], amask[:])
            iota = cp.tile([BC, V], F32)
            nc.sync.dma_start(iota[:], iota_f[:])
            hds = cp.tile([128, 8, 256], DT_MM)
            nc.sync.dma_start(hds[:], heads[:].rearrange("c p n -> p c n"))
            hdsb = cp.tile([1, 256], DT_MM)
            nc.sync.dma_start(hdsb[:], headsb[:])
            tpbt = cp.tile([V + 1, 4 * H], DT_MM)
            nc.sync.dma_start(tpbt[:], tpb[:])

            # persistent decode state
            hT_d = stp.tile([128, 4, BC], DT_MM)
            ctxT = stp.tile([128, 4, BC], DT_MM)
            ohT = stp.tile([V + 1, BC], DT_MM)
            c_d = stp.tile([BC, H], F32)
            qT_m = stp.tile([128, 4, QWIN], DT_MM)
            alphaT_m = stp.tile([128, WIN], DT_MM)
            nc.gpsimd.memset(qT_m[:].bitcast(dt.uint32), 0)
            nc.gpsimd.memset(hT_d[:].bitcast(dt.uint32), 0)
            nc.gpsimd.memset(alphaT_m[:].bitcast(dt.uint32), 0)
            nc.gpsimd.memset(c_d[:], 0.0)
            nc.sync.dma_start(ohT[:], bos_ohT[:])

            # ================= phase 1: x-projections =================
            with (
                tc.tile_pool(name="p1w", bufs=1) as p1w,
                tc.tile_pool(name="p1x", bufs=3) as p1x,
                tc.tile_pool(name="p1ps", bufs=3, space="PSUM") as p1ps,
            ):
                wf = p1w.tile([128, 6, 4 * HE], DT_MM)
                wb = p1w.tile([128, 6, 4 * HE], DT_MM)
                bf = p1w.tile([1, 4 * HE], DT_MM)
                bb = p1w.tile([1, 4 * HE], DT_MM)
                nc.sync.dma_start(wf[:], wihT_f[:].rearrange("c p n -> p c n"))
                nc.sync.dma_start(wb[:], wihT_b[:].rearrange("c p n -> p c n"))
                nc.sync.dma_start(bf[:], bias_f[:])
                nc.sync.dma_start(bb[:], bias_b[:])
                for b in range(BC):
                    xt = p1x.tile([128, 6, T], DT_MM, tag="xt")
                    nc.sync.dma_start(xt[:], xT[b].rearrange("c p t -> p c t"))
                    for w, bi, xp_d in ((wf, bf, xproj_f), (wb, bb, xproj_b)):
                        pg = p1ps.tile([128, 4 * HE], F32, tag="pg1")
                        for n in range(2):
                            nsl = slice(n * 512, n * 512 + 512)
                            for c in range(6):
                                nc.tensor.matmul(pg[:, nsl], xt[:, c, :], w[:, c, nsl],
                                                 start=(c == 0), stop=False)
                            nc.tensor.matmul(pg[:, nsl], onesr[:], bi[:, nsl],
                                             start=False, stop=True)
                        xps = p1x.tile([128, 4 * HE], DT_MM, tag="xps")
                        nc.vector.tensor_copy(xps[:], pg[:])
                        nc.sync.dma_start(xp_d[b], xps[:])

            # big enc tensors allocated after phase-1 weights are released
            if PHASES < 2:
                nc.compile()
                return nc
            with tc.tile_pool(name="big", bufs=1) as bigp:
                encT = bigp.tile([128, 4, BC, T], DT_MM)    # h-major enc
                enc_sb = bigp.tile([128, BC, H], DT_MM)     # t-major enc

                # ================= phase 2: encoder recurrence =================
                with (
                    tc.tile_pool(name="p2w", bufs=1) as p2w,
                    tc.tile_pool(name="p2s", bufs=1) as p2s,
                    tc.tile_pool(name="p2xp", bufs=3) as p2xp,
                    tc.tile_pool(name="p2t", bufs=2) as p2t,
                    tc.tile_pool(name="p2ps", bufs=2, space="PSUM") as p2ps,
                    tc.tile_pool(name="p2pt", bufs=2, space="PSUM") as p2pt,
                ):
                    whf = p2w.tile([128, 2, 4 * HE], DT_MM)
                    whb = p2w.tile([128, 2, 4 * HE], DT_MM)
                    nc.sync.dma_start(whf[:], whhT_f[:].rearrange("c p n -> p c n"))
                    nc.sync.dma_start(whb[:], whhT_b[:].rearrange("c p n -> p c n"))
                    sts = {}
                    for d in (0, 1):
                        hT_e = p2s.tile([128, 2, BC], DT_MM, tag=f"hTe{d}")
                        c_e = p2s.tile([BC, HE], F32, tag=f"ce{d}")
                        nc.gpsimd.memset(hT_e[:].bitcast(dt.uint32), 0)
                        nc.gpsimd.memset(c_e[:], 0.0)
                        sts[d] = (hT_e, c_e)
                    for s in range(T):
                        for d, wh, xp_d in ((0, whf, xproj_f), (1, whb, xproj_b)):
                            t = s if d == 0 else T - 1 - s
                            hT_e, c_e = sts[d]
                            xp = p2xp.tile([BC, 4 * HE], DT_MM, tag="xp")
                            nc.sync.dma_start(xp[:], xp_d[:, t, :])
                            pg = p2ps.tile([BC, 4 * HE], F32, tag="pg2")
                            for n in range(2):
                                nsl = slice(n * 512, n * 512 + 512)
                                nc.tensor.matmul(pg[:, nsl], ident[:BC, :BC], xp[:, nsl],
                                                 start=True, stop=False)
                                for c in range(2):
                                    nc.tensor.matmul(pg[:, nsl], hT_e[:, c, :],
                                                     wh[:, c, nsl],
                                                     start=False, stop=(c == 1))
                            sg_if = p2t.tile([BC, 2 * HE], F32, tag="sgif")
                            nc.scalar.activation(sg_if[:], pg[:, 0:2 * HE], AF.Sigmoid)
                            tg = p2t.tile([BC, HE], F32, tag="tg")
                            nc.scalar.activation(tg[:], pg[:, 2 * HE:3 * HE], AF.Tanh)
                            so = p2t.tile([BC, HE], F32, tag="so")
                            nc.scalar.activation(so[:], pg[:, 3 * HE:4 * HE], AF.Sigmoid)
                            t1 = p2t.tile([BC, HE], F32, tag="t1")
                            nc.vector.tensor_mul(t1[:], sg_if[:, HE:2 * HE], c_e[:])
                            t2 = p2t.tile([BC, HE], F32, tag="t2")
                            nc.vector.tensor_mul(t2[:], sg_if[:, 0:HE], tg[:])
                            nc.vector.tensor_add(c_e[:], t1[:], t2[:])
                            thc = p2t.tile([BC, HE], F32, tag="tg")
                            nc.scalar.activation(thc[:], c_e[:], AF.Tanh)
                            h_sb = p2t.tile([BC, HE], DT_MM, tag="hsb")
                            nc.vector.tensor_mul(h_sb[:], so[:], thc[:])
                            pt = p2pt.tile([128, 64], DT_MM, tag="pt2")
                            for c in range(2):
                                nc.tensor.transpose(pt[:, c * 32:c * 32 + 32],
                                                    h_sb[:, c * 128:c * 128 + 128],
                                                    ident[:BC, :BC])
                            nc.vector.tensor_copy(
                                hT_e[:], pt[:].rearrange("p (c m) -> p c m", c=2))
                            for c in range(2):
                                nc.vector.tensor_copy(encT[:, 2 * d + c, :, t],
                                                      hT_e[:, c, :])

                # ======== phase 3: build enc_sb (t-major) from encT ========
                with tc.tile_pool(name="p3pt", bufs=6, space="PSUM") as p3pt:
                  if PHASES >= 3:
                    for b in range(BC):
                        for c in range(4):
                            pt = p3pt.tile([128, 128], DT_MM, tag="p3")
                            nc.tensor.transpose(pt[:], encT[:, c, b, :], ident[:])
                            nc.vector.tensor_copy(
                                enc_sb[:, b, c * 128:c * 128 + 128], pt[:])

                if PHASES < 4:
                    nc.compile()
                    return nc
                # ======== phase 4: ctx_init = enc[b, count_b] ========
                with (
                    tc.tile_pool(name="p4", bufs=1) as p4,
                    tc.tile_pool(name="p4ps", bufs=1, space="PSUM") as p4ps,
                    tc.tile_pool(name="p4pt", bufs=1, space="PSUM") as p4pt,
                ):
                    cntm = p4.tile([128, WIN], DT_MM)
                    nc.sync.dma_start(cntm[:], cnt_m[:])
                    pci = p4ps.tile([BC, H], F32)
                    for b in range(BC):
                        nc.tensor.matmul(pci[:, :], cntm[:, 33 * b:33 * b + 32],
                                         enc_sb[:, b, :], start=(b == 0),
                                         stop=(b == BC - 1))
                    ci_sb = p4.tile([BC, H], DT_MM)
                    nc.scalar.activation(ci_sb[:], pci[:], AF.Copy)
                    ptc = p4pt.tile([128, 128], DT_MM)
                    for c in range(4):
                        nc.tensor.transpose(ptc[:, c * 32:c * 32 + 32],
                                            ci_sb[:, c * 128:c * 128 + 128],
                                            ident[:BC, :BC])
                    nc.vector.tensor_copy(ctxT[:], ptc[:].rearrange("p (c m) -> p c m", c=4))

                if PHASES < 5:
                    nc.compile()
                    return nc
                # ======== phase 5: A3 = enc @ W3.T to DRAM ========
                with (
                    tc.tile_pool(name="p5w", bufs=2) as p5w,
                    tc.tile_pool(name="p5ps", bufs=6, space="PSUM") as p5ps,
                ):
                    for nh in range(2):
                        w3 = p5w.tile([128, 4, 2 * H], DT_MM, tag="w3")
                        nc.sync.dma_start(
                            w3[:], w3T[:, :, nh * 1024:nh * 1024 + 1024]
                            .rearrange("c p n -> p c n"))
                        for b in range(BC):
                            for n in range(2):
                                off = nh * 1024 + n * 512
                                pa = p5ps.tile([128, 512], F32, tag="pa")
                                for c in range(4):
                                    nc.tensor.matmul(
                                        pa[:], encT[:, c, b, :],
                                        w3[:, c, n * 512:n * 512 + 512],
                                        start=(c == 0), stop=(c == 3))
                                pas = p5w.tile([128, 512], DT_MM, tag="pas")
                                nc.vector.tensor_copy(pas[:], pa[:])
                                nc.sync.dma_start(a3[:, b, off:off + 512], pas[:])

                if PHASES < 6:
                    nc.compile()
                    return nc
                # ================= phase 6: decode loop =================
                with (
                    tc.tile_pool(name="wdxp", bufs=4) as wdxp,
                    tc.tile_pool(name="a3p", bufs=2) as a3p,
                    tc.tile_pool(name="dtp", bufs=1) as dtp,
                    tc.tile_pool(name="dsm", bufs=2) as dsm,
                    tc.tile_pool(name="dps", bufs=2, space="PSUM") as dps,
                    tc.tile_pool(name="dpg", bufs=1, space="PSUM") as dpg,
                    tc.tile_pool(name="dpt", bufs=2, space="PSUM") as dpt,
                ):
                    for t in range(T):
                        # ---- gates ----
                        pg = dpg.tile([BC, 4 * H], F32, tag="pg")
                        for half in range(2):
                            a3t = a3p.tile([BC, 2 * H], DT_MM, tag="a3t")
                            nc.sync.dma_start(
                                a3t[:], a3[t, :, half * 1024:half * 1024 + 1024])
                            for n in range(2):
                                o = half * 1024 + n * 512
                                nc.tensor.matmul(pg[:, o:o + 512], ident[:BC, :BC],
                                                 a3t[:, n * 512:n * 512 + 512],
                                                 start=True, stop=False)
                        for c in range(8):
                            lhs = hT_d[:, c, :] if c < 4 else ctxT[:, c - 4, :]
                            for half in range(2):
                                wt = wdxp.tile([128, 2 * H], DT_MM, tag="wdx")
                                nc.sync.dma_start(
                                    wt[:], wdx[c, :, half * 1024:half * 1024 + 1024])
                                for n in range(2):
                                    o = half * 1024 + n * 512
                                    nc.tensor.matmul(pg[:, o:o + 512], lhs,
                                                     wt[:, n * 512:n * 512 + 512],
                                                     start=False, stop=False)
                        for n in range(4):
                            nc.tensor.matmul(pg[:, n * 512:n * 512 + 512], ohT[:],
                                             tpbt[:, n * 512:n * 512 + 512],
                                             start=False, stop=True)
                        # ---- pointwise ----
                        sg_if = dtp.tile([BC, 2 * H], F32, tag="sgif")
                        nc.scalar.activation(sg_if[:], pg[:, 0:2 * H], AF.Sigmoid)
                        tg = dtp.tile([BC, H], F32, tag="tg")
                        nc.scalar.activation(tg[:], pg[:, 2 * H:3 * H], AF.Tanh)
                        so = dtp.tile([BC, H], F32, tag="so")
                        nc.scalar.activation(so[:], pg[:, 3 * H:4 * H], AF.Sigmoid)
                        t1 = dtp.tile([BC, H], F32, tag="t1")
                        nc.vector.tensor_mul(t1[:], sg_if[:, H:2 * H], c_d[:])
                        t2 = dtp.tile([BC, H], F32, tag="t2")
                        nc.vector.tensor_mul(t2[:], sg_if[:, 0:H], tg[:])
                        nc.vector.tensor_add(c_d[:], t1[:], t2[:])
                        thc = dtp.tile([BC, H], F32, tag="tg")
                        nc.scalar.activation(thc[:], c_d[:], AF.Tanh)
                        h_sb = dtp.tile([BC, H], DT_MM, tag="hsb")
                        nc.vector.tensor_mul(h_sb[:], so[:], thc[:])
                        pth = dpt.tile([128, 128], DT_MM, tag="pth")
                        for c in range(4):
                            nc.tensor.transpose(pth[:, c * 32:c * 32 + 32],
                                                h_sb[:, c * 128:c * 128 + 128],
                                                ident[:BC, :BC])
                        nc.vector.tensor_copy(
                            hT_d[:], pth[:].rearrange("p (c m) -> p c m", c=4))

                        # ---- attention ----
                        pq = dps.tile([BC, 512], F32, tag="pqs")
                        for half in range(2):
                            wq = wdxp.tile([128, 2 * H], DT_MM, tag="wdx")
                            nc.sync.dma_start(
                                wq[:], wdx[8, :, half * 1024:half * 1024 + 1024])
                            for c2 in range(2):
                                c = half * 2 + c2
                                nc.tensor.matmul(pq[:], hT_d[:, c, :],
                                                 wq[:, c2 * 512:c2 * 512 + 512],
                                                 start=(c == 0), stop=(c == 3))
                        q_sb = dtp.tile([BC, H], DT_MM, tag="qsb")
                        nc.scalar.activation(q_sb[:], pq[:], AF.Copy)
                        ptq = dpt.tile([128, 128], DT_MM, tag="pth")
                        for c in range(4):
                            nc.tensor.transpose(ptq[:, c * 32:c * 32 + 32],
                                                q_sb[:, c * 128:c * 128 + 128],
                                                ident[:BC, :BC])
                        ptqv = ptq[:].rearrange("p (c m) -> p c m", c=4)
                        for g in range(8):
                            nc.vector.tensor_copy(qT_m[:, :, 40 * g:40 * g + 4],
                                                  ptqv[:, :, 4 * g:4 * g + 4])
                        pe = dps.tile([BC, 512], F32, tag="pqs")
                        for g in range(8):
                            for c in range(4):
                                nc.tensor.matmul(pe[:], qT_m[:, c, 36 * g:36 * g + 32],
                                                 encT[:, c, 4 * g:4 * g + 4, :],
                                                 start=(g == 0 and c == 0),
                                                 stop=(g == 7 and c == 3))
                        me = dtp.tile([BC, 512], F32, tag="sgif")
                        nc.vector.tensor_add(me[:], pe[:], msk[:])
                        mxa = dsm.tile([BC, 1], F32, tag="mxa")
                        nc.vector.tensor_reduce(out=mxa[:], in_=me[:],
                                                axis=mybir.AxisListType.X, op=ALU.max)
                        ngmx = dsm.tile([BC, 1], F32, tag="ngmx")
                        nc.vector.tensor_scalar_mul(ngmx[:], mxa[:], -1.0)
                        alpha = dtp.tile([BC, 512], DT_MM, tag="tg")
                        asum = dsm.tile([BC, 1], F32, tag="asum")
                        nc.scalar.activation(alpha[:], me[:], AF.Exp, bias=ngmx[:],
                                             scale=1.0, accum_out=asum[:])
                        rsum = dsm.tile([BC, 1], F32, tag="rsum")
                        nc.vector.reciprocal(rsum[:], asum[:])
                        pta = dpt.tile([128, 128], DT_MM, tag="pth")
                        for c in range(4):
                            nc.tensor.transpose(pta[:, c * 32:c * 32 + 32],
                                                alpha[:, c * 128:c * 128 + 128],
                                                ident[:BC, :BC])
                        # alphaT_m[p, 136g+34j] = pta[p, 33j+4g]  (b = 4g+j)
                        nc.vector.tensor_copy(
                            _winap(alphaT_m[:], [(136, 8), (34, 4)], 128, WIN),
                            _winap(pta[:], [(4, 8), (33, 4)], 128, 128))
                        pc = dps.tile([BC, 512], F32, tag="pqs")
                        for b in range(BC):
                            nc.tensor.matmul(pc[:], alphaT_m[:, 33 * b:33 * b + 32],
                                             enc_sb[:, b, :], start=(b == 0),
                                             stop=(b == BC - 1))
                        ctx_sb = dtp.tile([BC, H], DT_MM, tag="so")
                        nc.scalar.activation(ctx_sb[:], pc[:], AF.Copy, scale=rsum[:])
                        ptc = dpt.tile([128, 128], DT_MM, tag="pth")
                        for c in range(4):
                            nc.tensor.transpose(ptc[:, c * 32:c * 32 + 32],
                                                ctx_sb[:, c * 128:c * 128 + 128],
                                                ident[:BC, :BC])
                        # ---- scores (slot head, ctxT = ctx_{t-1}) ----
                        ps = dps.tile([BC, 512], F32, tag="pqs")
                        for c in range(4):
                            nc.tensor.matmul(ps[:, 0:256], hT_d[:, c, :], hds[:, c, :],
                                             start=(c == 0), stop=False)
                        for c in range(4):
                            nc.tensor.matmul(ps[:, 0:256], ctxT[:, c, :],
                                             hds[:, 4 + c, :], start=False, stop=False)
                        nc.tensor.matmul(ps[:, 0:256], onesr[:, :BC], hdsb[:],
                                         start=False, stop=True)
                        scr = dsm.tile([BC, 184], F32, tag="scr")
                        nc.scalar.activation(scr[:], ps[:, 0:184], AF.Copy)
                        # argmax + one-hot (feeds next step's gates)
                        mx8 = dsm.tile([BC, 8], F32, tag="mx8")
                        nc.vector.max(mx8[:], scr[:, 0:SLOT])
                        idx8 = dsm.tile([BC, 8], dt.uint16, tag="idx8")
                        nc.vector.max_index(idx8[:], mx8[:], scr[:, 0:SLOT])
                        idxf = dsm.tile([BC, 1], F32, tag="idxf")
                        nc.vector.tensor_copy(idxf[:], idx8[:, 0:1])
                        oh = dsm.tile([BC, V], DT_MM, tag="oh")
                        nc.vector.tensor_scalar(out=oh[:], in0=iota[:], scalar1=idxf[:],
                                                scalar2=None, op0=ALU.is_equal)
                        poh = dpt.tile([128, 128], DT_MM, tag="pth")
                        nc.tensor.transpose(poh[0:V, 0:BC], oh[:], ident[:BC, :BC])
                        nc.vector.tensor_copy(ohT[0:V, :], poh[0:V, 0:BC])
                        # log-softmax -> slot scores out
                        nmx = dsm.tile([BC, 1], F32, tag="nmx")
                        nc.vector.tensor_scalar_mul(nmx[:], mx8[:, 0:1], -1.0)
                        expv = dtp.tile([BC, H], F32, tag="t2")
                        sume = dsm.tile([BC, 1], F32, tag="sume")
                        nc.scalar.activation(expv[:, 0:SLOT], scr[:, 0:SLOT], AF.Exp,
                                             bias=nmx[:], scale=1.0, accum_out=sume[:])
                        lse = dsm.tile([BC, 1], F32, tag="lse")
                        nc.scalar.activation(lse[:], sume[:], AF.Ln)
                        nb = dsm.tile([BC, 1], F32, tag="nb")
                        nc.vector.tensor_sub(nb[:], nmx[:], lse[:])
                        logp = dtp.tile([BC, H], F32, tag="t1")
                        nc.scalar.activation(logp[:, 0:SLOT], scr[:, 0:SLOT],
                                             AF.Identity, bias=nb[:], scale=1.0)
                        nc.sync.dma_start(out_slot[:, t, :], logp[:, 0:SLOT])

                        nc.vector.tensor_copy(
                            ctxT[:], ptc[:].rearrange("p (c m) -> p c m", c=4))

                        if t == 0:
                            pi = dps.tile([BC, 512], F32, tag="pqs")
                            for c in range(4):
                                nc.tensor.matmul(pi[:, 0:256], hT_d[:, c, :],
                                                 hds[:, c, :], start=(c == 0),
                                                 stop=False)
                            for c in range(4):
                                nc.tensor.matmul(pi[:, 0:256], ctxT[:, c, :],
                                                 hds[:, 4 + c, :],
                                                 start=False, stop=False)
                            nc.tensor.matmul(pi[:, 0:256], onesr[:, :BC], hdsb[:],
                                             start=False, stop=True)
                            intn = dsm.tile([BC, INTENT], F32, tag="intn")
                            nc.scalar.activation(intn[:], pi[:, SLOT:SLOT + INTENT],
                                                 AF.Copy)
                            nc.sync.dma_start(out_intent[:], intn[:])
    nc.compile()
    return nc


def _round_f32r(x):
    # placeholder: host-side rounding to f32r-representable values (identity
    # until HW rounding semantics are confirmed)
    return np.ascontiguousarray(x, dtype=np.float32)


def _prep_core(sl, a):
    f32 = np.float32
    x = np.ascontiguousarray(a["seq_reps"][sl]).astype(f32)       # [BC,T,E]
    xT = np.ascontiguousarray(x.transpose(0, 2, 1)).reshape(BC, 6, 128, T)
    Wd_ih, Wd_hh = a["Wd_ih"], a["Wd_hh"]
    W1 = Wd_ih[:, 0:EMB]
    W2 = Wd_ih[:, EMB:EMB + H]
    W3 = Wd_ih[:, EMB + H:]
    wdx = np.zeros((9, 128, 4 * H), f32)
    wdx[0:4] = Wd_hh.T.reshape(4, 128, 4 * H)
    wdx[4:8] = W2.T.reshape(4, 128, 4 * H)
    # chunk 8: attn_W row-chunks packed side by side [128, 4*512]
    aw = a["attn_W"].reshape(4, 128, H)
    wdx[8] = np.concatenate([aw[c] for c in range(4)], axis=1)
    tpbm = np.concatenate([a["emb_table"] @ W1.T,
                           (a["bd_ih"] + a["bd_hh"])[None, :]], 0)
    hd = np.zeros((1025, 256), f32)
    hd[0:1024, 0:SLOT] = a["slot_W"].T
    hd[0:1024, SLOT:SLOT + INTENT] = a["intent_W"].T
    hd[1024, 0:SLOT] = a["slot_b"]
    hd[1024, SLOT:SLOT + INTENT] = a["intent_b"]
    bos_ohT = np.zeros((V + 1, BC), f32)
    bos_ohT[SLOT, :] = 1.0
    bos_ohT[V, :] = 1.0
    cnt = np.clip((a["nwp_index"][sl, :, 0] != 0).sum(1), 0, T - 1)
    cnt_m = np.zeros((128, WIN), f32)
    for b in range(BC):
        cnt_m[int(cnt[b]), 34 * b] = 1.0
    som = a["slot_output_mask"][sl]                               # [BC,T] bool
    amask = np.full((BC, 512), NEG, f32)
    for b in range(BC):
        j = b % 4
        amask[b, j * 128:(j + 1) * 128] = np.where(som[b], NEG, 0.0)
    r = _round_f32r
    return {
        "xT": r(xT),
        "wihT_f": r(a["Wf_ih"].T.reshape(6, 128, 4 * HE)),
        "wihT_b": r(a["Wb_ih"].T.reshape(6, 128, 4 * HE)),
        "bias_f": r((a["bf_ih"] + a["bf_hh"]).reshape(1, 4 * HE)),
        "bias_b": r((a["bb_ih"] + a["bb_hh"]).reshape(1, 4 * HE)),
        "whhT_f": r(a["Wf_hh"].T.reshape(2, 128, 4 * HE)),
        "whhT_b": r(a["Wb_hh"].T.reshape(2, 128, 4 * HE)),
        "wdx": r(wdx), "tpb": r(tpbm),
        "w3T": r(W3.T.reshape(4, 128, 4 * H)),
        "heads": r(hd[0:1024].reshape(8, 128, 256)),
        "headsb": r(hd[1024:1025]),
        "identD": np.eye(128, dtype=f32),
        "ones1": np.ones((1, 128), f32),
        "bos_ohT": r(bos_ohT), "cnt_m": r(cnt_m),
        "amask": amask,
        "iota_f": np.broadcast_to(np.arange(V, dtype=f32), (BC, V)).copy(),
    }


def get_nc():
    if "nc" not in _CACHE:
        _CACHE["nc"] = _build(bacc.Bacc("TRN2", target_bir_lowering=False, debug=False))
    return _CACHE["nc"]


def make_in_maps(**inputs):
    args = {k: np.asarray(v) for k, v in inputs.items()}
    return [_prep_core(slice(c * BC, (c + 1) * BC), args) for c in range(NCORES)]


def _get_exec():
    if "exec" in _CACHE:
        return _CACHE["exec"]
    import jax
    from jax.sharding import Mesh, PartitionSpec, NamedSharding
    from jax.experimental.shard_map import shard_map
    from concourse.bass2jax import (_bass_exec_p, install_neuronx_cc_hook,
                                    partition_id_tensor)
    nc = get_nc()
    install_neuronx_cc_hook()
    pname = nc.partition_id_tensor.name if nc.partition_id_tensor else None
    in_names, out_names, out_avals, zero_shapes = [], [], [], []
    for alloc in nc.m.functions[0].allocations:
        if not isinstance(alloc, mybir.MemoryLocationSet):
            continue
        name = alloc.memorylocations[0].name
        if alloc.kind == "ExternalInput":
            if name != pname:
                in_names.append(name)
        elif alloc.kind == "ExternalOutput":
            out_names.append(name)
            shape = tuple(alloc.tensor_shape)
            npdt = mybir.dt.np(alloc.dtype)
            out_avals.append(jax.core.ShapedArray(shape, npdt))
            zero_shapes.append((shape, npdt))
    n_params, n_outs = len(in_names), len(out_avals)
    all_in = list(in_names) + out_names + ([pname] if pname else [])

    def _body(*args):
        operands = list(args)
        if pname is not None:
            operands.append(partition_id_tensor())
        return tuple(_bass_exec_p.bind(
            *operands, out_avals=tuple(out_avals), in_names=tuple(all_in),
            out_names=tuple(out_names), lowering_input_output_aliases=(),
            sim_require_finite=True, sim_require_nnan=True, nc=nc))

    mesh = Mesh(np.asarray(jax.devices()[:NCORES]), ("core",))
    sharded = jax.jit(
        shard_map(_body, mesh=mesh,
                  in_specs=(PartitionSpec("core"),) * (n_params + n_outs),
                  out_specs=(PartitionSpec("core"),) * n_outs, check_rep=False),
        donate_argnums=tuple(range(n_params, n_params + n_outs)), keep_unused=True)
    sh = NamedSharding(mesh, PartitionSpec("core"))
    _CACHE["exec"] = (sharded, sh, in_names, out_names, zero_shapes)
    return _CACHE["exec"]


def kernel(**inputs):
    import jax
    sharded, sh, in_names, out_names, zero_shapes = _get_exec()
    in_maps = make_in_maps(**inputs)
    dev_in = [jax.device_put(np.concatenate(
        [np.asarray(in_maps[c][nm]) for c in range(NCORES)], 0), sh)
        for nm in in_names]
    zeros = [jax.device_put(np.zeros((NCORES * s0[0], *s0[1:]), d0), sh)
             for s0, d0 in zero_shapes]
    outs = sharded(*dev_in, *zeros)
    jax.block_until_ready(outs)
    res = {nm: np.asarray(outs[i]) for i, nm in enumerate(out_names)}
    slot_scores = res["out_slot"].reshape(NCORES * BC, T, SLOT)
    intent_score = res["out_intent"].reshape(NCORES * BC, INTENT)
    return slot_scores.astype(np.float32), intent_score.astype(np.float32)
